# revision 1
# baseline (speedup 1.0000x reference)
"""Causal multi-head attention (8 heads, 1x1-conv projections) on 8 TRN2 cores.

Sharding: data-parallel over batch N=8 -> one batch element per NeuronCore.
Per-core kernel (S=1024 pixels, C=E=256 channels, H=8 heads, d=32):
  q = WqT.T @ x, k = WkT.T @ x              (e, s) layout, fp32r matmuls
  vT = x.T @ WvT                            (s, e) layout (transposed v, so the
                                            attention contraction needs no
                                            on-chip transpose of big tensors)
  per head: P^T[sk, sq] = exp(mask(k_h^T q_h))   scores computed TRANSPOSED so
                                            softmax denominator comes from an
                                            appended ones-column in v (M=33)
  out_h = (vAug_h^T @ P^T) -> rows 0..31 numerator^T, row 32 = denominator
  att = num / denom (per-column broadcast via DRAM-bounce DMA broadcast)
  out = WprojT.T @ att + bproj_eff
Host folds: 1/sqrt(d) into Wq/bq; v-bias through the projection
(bproj_eff = bproj + Wproj @ bv, valid because attention rows sum to 1).
P^T and v^T are bf16 (fp32 accumulate); projections and scores are fp32r.
Emission is software-pipelined: PE warmup matmuls run during the input DMA,
head h+1's first scores chunk is emitted before head h's last (so the ACT
exp stream - the bottleneck engine - never stalls at head boundaries), and
each head's attn@v/normalization is deferred into the next head's scores.
The softmax normalization broadcasts 1/denominator across partitions via a
DRAM-bounce DMA (SBUF reads cannot replicate partitions); the final head
uses a DMA-free chain (PE outer-product broadcast) to shorten the tail.
"""

import numpy as np

N_CORES = 8
C = 256      # input channels
E = 256      # embed channels (q/k)
O = 256      # v/out channels
S = 1024     # spatial positions (32*32)
H = 8        # heads
D = 32       # head dim
NCH = 2      # 256 = 2 * 128 partition chunks

_CACHE = {}


def _build_program():
    import concourse.bass as bass
    import concourse.mybir as mybir
    from concourse import bacc
    from concourse.tile import TileContext

    F32 = mybir.dt.float32
    F32R = mybir.dt.float32r
    BF16 = mybir.dt.bfloat16
    EXP = mybir.ActivationFunctionType.Exp

    nc = bacc.Bacc("TRN2", target_bir_lowering=False, debug=False)

    # fp32r inputs: DMA is an accepted f32r producer, PE rounds on read
    xin = nc.dram_tensor("xin", [C, S], F32R, kind="ExternalInput")
    wqt = nc.dram_tensor("wqt", [C, E], F32R, kind="ExternalInput")
    wkt = nc.dram_tensor("wkt", [C, E], F32R, kind="ExternalInput")
    wvt = nc.dram_tensor("wvt", [C, O], F32R, kind="ExternalInput")
    wpt = nc.dram_tensor("wpt", [O, O], F32R, kind="ExternalInput")
    onesd = nc.dram_tensor("onesd", [8, H], BF16, kind="ExternalInput")
    onr = nc.dram_tensor("onr", [1, 32], F32R, kind="ExternalInput")
    bqd = nc.dram_tensor("bqd", [E], F32, kind="ExternalInput")
    bkd = nc.dram_tensor("bkd", [E], F32, kind="ExternalInput")
    bpd = nc.dram_tensor("bpd", [O], F32, kind="ExternalInput")
    outd = nc.dram_tensor("out", [O, S], F32, kind="ExternalOutput")
    # scratch for the denominator broadcast bounce: (head, j, sq)
    dscr = nc.dram_tensor("dscr", [H, 2, 512], F32)

    with TileContext(nc) as tc:
        with (
            tc.tile_pool(name="cst", bufs=1) as cst,
            tc.tile_pool(name="ptp", bufs=3) as ptp,
            tc.tile_pool(name="rbp", bufs=4) as rbp,
            tc.tile_pool(name="osb", bufs=2) as osb,
            tc.tile_pool(name="psc", bufs=2, space="PSUM") as psc,
            tc.tile_pool(name="pav", bufs=4, space="PSUM") as pav,
        ):
            # --- preload exp table + warm the PE clock while DMAs run ---
            dmz = cst.tile([128, 64], F32, tag="dmz")
            nc.vector.memset(dmz, 0.0)
            dme = cst.tile([128, 1], F32, tag="dme")
            nc.scalar.activation(dme, dmz[:, 0:1], EXP)
            wup = cst.tile([128, 64], F32R, tag="wup")
            nc.vector.tensor_copy(wup, dmz)
            pwu = pav.tile([64, 512], F32, tag="pa")
            for _ in range(36):
                nc.tensor.matmul(pwu[:, 0:64], wup, wup[:, 0:64].bitcast(F32R),
                                 start=True, stop=True)

            # --- load order follows the critical path: q/k weights + biases
            # (small, gate the first projection), then the first x half, then
            # everything else (DMA transfers serialize on the engine pool) ---
            wtiles = {}
            btiles = {}

            def wload(name, dram):
                wr = cst.tile([128, NCH, 256], F32R, tag=name + "r")
                nc.sync.dma_start(out=wr, in_=dram.ap().rearrange("(c p) e -> p c e", p=128))
                wtiles[name] = wr

            def bload(name, dram):
                bt = cst.tile([128, NCH], F32, tag=name + "t")
                nc.sync.dma_start(out=bt, in_=dram.ap().rearrange("(m p) -> p m", p=128))
                btiles[name] = bt

            wload("wq", wqt)
            wload("wk", wkt)
            bload("bq", bqd)
            bload("bk", bkd)
            xr = cst.tile([128, NCH, S], F32R, tag="xr")
            xsrc = xin.ap().rearrange("(c p) s -> p c s", p=128)
            for jh in range(2):
                nc.sync.dma_start(
                    out=xr[:, :, jh * 512:(jh + 1) * 512],
                    in_=xsrc[:, :, jh * 512:(jh + 1) * 512],
                )
            wload("wv", wvt)
            wload("wp", wpt)
            bload("bp", bpd)

            # v^T augmented with a ones column per head: (sk_part, chunk, head, 33)
            vaug = cst.tile([128, 8, H, D + 1], BF16, tag="vaug")
            oap = onesd.ap()
            ones_bcast = bass.AP(
                tensor=oap.tensor, offset=oap.offset, ap=[[0, 128]] + list(oap.ap)
            )
            nc.sync.dma_start(out=vaug[:, :, :, D], in_=ones_bcast)

            onc = cst.tile([33, 32], F32R, tag="onc")
            nc.sync.dma_start(out=onc[32:33, :], in_=onr[:])

            q_sb = cst.tile([128, NCH, S], F32R, tag="q_sb")
            k_sb = cst.tile([128, NCH, S], F32R, tag="k_sb")
            att = cst.tile([128, NCH, S], F32R, tag="att")

            def qk_proj_unit(wname, bname, dst, m, j):
                wr, bt = wtiles[wname], btiles[bname]
                pp = psc.tile([128, 512], F32, tag="sc")
                for c in range(2):
                    nc.tensor.matmul(
                        pp,
                        wr[:, c, m * 128:(m + 1) * 128],
                        xr[:, c, j * 512:(j + 1) * 512],
                        start=(c == 0), stop=(c == 1),
                    )
                nc.vector.tensor_scalar_add(
                    dst[:, m, j * 512:(j + 1) * 512], pp, bt[:, m:m + 1]
                )

            def qk_proj(m):
                for wname, bname, dst in (("wq", "bq", q_sb), ("wk", "bk", k_sb)):
                    for j in range(2):
                        qk_proj_unit(wname, bname, dst, m, j)

            def v_proj_unit(i):
                pv = psc.tile([128, 512], F32, tag="sc")
                wr = wtiles["wv"]
                for c in range(2):
                    nc.tensor.matmul(
                        pv[:, 0:256],
                        xr[:, c, i * 128:(i + 1) * 128],
                        wr[:, c, :],
                        start=(c == 0), stop=(c == 1),
                    )
                nc.vector.tensor_copy(
                    vaug[:, i, :, 0:D],
                    pv[:, 0:256].rearrange("p (h d) -> p h d", h=H),
                )

            def scores_chunk(h, pts, i, split=False):
                m, r = h // 4, h % 4
                rows = slice(32 * r, 32 * r + 32)
                ps = psc.tile([128, S], F32, tag="sc")
                for j in range(2):
                    if 512 * (j + 1) <= 128 * i:
                        continue
                    ws = max(512 * j, 128 * i)
                    we = 512 * (j + 1)
                    nc.tensor.matmul(
                        ps[:, ws:we],
                        k_sb[rows, m, 128 * i:128 * (i + 1)],
                        q_sb[rows, m, ws:we],
                        start=True, stop=True,
                        tile_position=(32 * r, 0),
                    )
                    if split:
                        # per-window exp: lets ACT start on the j=0 half while
                        # the j=1 projections/scores are still in flight
                        nc.scalar.activation(pts[:, i, ws:we], ps[:, ws:we], EXP)
                if not split:
                    nc.scalar.activation(pts[:, i, 128 * i:S], ps[:, 128 * i:S], EXP)
                # keep where sq_local - sk_local >= 0 in the diagonal block
                nc.gpsimd.affine_select(
                    out=pts[:, i, 128 * i:128 * (i + 1)],
                    in_=pts[:, i, 128 * i:128 * (i + 1)],
                    compare_op=mybir.AluOpType.is_ge,
                    fill=0.0,
                    base=0,
                    channel_multiplier=-1,
                    pattern=[[1, 128]],
                )

            def attnv(h, pts, j, fast=False):
                # attn@v for sq-half j; own 1-bank psum slot so the slow
                # normalization chain never blocks the scores/exp pipeline
                m, r = h // 4, h % 4
                pa = pav.tile([33, 512], F32, tag="pa")
                ii = [i for i in range(8) if 128 * i < 512 * (j + 1)]
                for idx, i in enumerate(ii):
                    ws = max(512 * j, 128 * i)
                    we = 512 * (j + 1)
                    nc.tensor.matmul(
                        pa[:, ws - 512 * j:we - 512 * j],
                        vaug[:, i, h, :],
                        pts[:, i, ws:we],
                        start=(idx == 0), stop=(idx == len(ii) - 1),
                    )
                if fast:
                    # tail chain: recip -> PE outer-product broadcast -> DVE
                    # copy to SBUF -> mul (no DMA latency)
                    rf = rbp.tile([33, 512], F32R, tag="rff")
                    with nc.allow_low_precision(reason="softmax recip in f32r"):
                        nc.vector.reciprocal(rf, pa)
                    pb = pav.tile([32, 512], F32, tag="pa")
                    nc.tensor.matmul(pb, onc[32:33, :], rf[32:33, :],
                                     start=True, stop=True)
                    rb = rbp.tile([32, 512], F32, tag="rb")
                    nc.vector.tensor_copy(rb, pb)
                else:
                    rf = rbp.tile([33, 512], F32, tag="rf")
                    nc.vector.reciprocal(rf, pa)
                    nc.sync.dma_start(out=dscr[h, j], in_=rf[32:33, :])
                    rb = rbp.tile([32, 512], F32, tag="rb")
                    dap = dscr.ap()
                    nc.gpsimd.dma_start(
                        out=rb,
                        in_=bass.AP(
                            tensor=dap.tensor,
                            offset=dap.offset + (h * 2 + j) * 512,
                            ap=[[0, 32], [1, 512]],
                        ),
                    )
                nc.vector.tensor_mul(
                    att[32 * r:32 * r + 32, m, 512 * j:512 * (j + 1)],
                    pa[0:32, :], rb,
                )

            def outproj_unit(m, j):
                wr, bt = wtiles["wp"], btiles["bp"]
                po = psc.tile([128, 512], F32, tag="sc")
                for c in range(2):
                    nc.tensor.matmul(
                        po,
                        wr[:, c, m * 128:(m + 1) * 128],
                        att[:, c, j * 512:(j + 1) * 512],
                        start=(c == 0), stop=(c == 1),
                    )
                ot = osb.tile([128, 512], F32, tag="ot")
                nc.vector.tensor_scalar_add(ot, po, bt[:, m:m + 1])
                nc.sync.dma_start(
                    out=outd.ap().rearrange("(m p) s -> p m s", p=128)[
                        :, m, j * 512:(j + 1) * 512
                    ],
                    in_=ot,
                )

            # software-pipelined emission with one-chunk lookahead: head h+1's
            # first scores chunk is emitted before head h's last, so ACT never
            # waits at head boundaries; attn@v halves are deferred one head
            qk_proj(0)
            pts_tiles = {}

            def sc(h, i):
                if h not in pts_tiles:
                    pts = ptp.tile([128, 8, S], BF16, tag="pts")
                    pts_tiles[h] = pts
                scores_chunk(h, pts_tiles[h], i)

            sc(0, 0)
            for h in range(H):
                last = h == H - 1
                if h > 0:
                    attnv(h - 1, pts_tiles[h - 1], 0)
                for i in range(1, 8):
                    if i == 3 and h > 0:
                        attnv(h - 1, pts_tiles[h - 1], 1)
                        pts_tiles.pop(h - 1)
                    if i == 6 and last:
                        attnv(h, pts_tiles[h], 0, fast=True)
                    if i == 7 and not last:
                        sc(h + 1, 0)
                    sc(h, i)
                    if h == 0 and 1 <= i <= 4:
                        v_proj_unit(2 * (i - 1))
                        v_proj_unit(2 * (i - 1) + 1)
                    if h == 1 and 1 <= i <= 4:
                        qk_proj_unit(("wq", "wk")[i % 2], ("bq", "bk")[i % 2],
                                     (q_sb, k_sb)[i % 2], 1, (i - 1) // 2)
            # the j=0 output-projection half only needs the j=0 columns of
            # att, which are complete before the last head's second half
            outproj_unit(0, 0)
            outproj_unit(1, 0)
            attnv(H - 1, pts_tiles[H - 1], 1, fast=True)
            outproj_unit(0, 1)
            outproj_unit(1, 1)


    nc.compile()
    return nc


def get_program():
    if "nc" not in _CACHE:
        _CACHE["nc"] = _build_program()
    return _CACHE["nc"]


def kernel(x, wq, bq, wkv, bkv, wproj, bproj):
    import ml_dtypes
    from concourse.bass_utils import run_bass_kernel_spmd

    nc = get_program()

    x = np.asarray(x, dtype=np.float32)
    n = x.shape[0]
    assert n == N_CORES and x.shape[1:] == (C, 32, 32)

    scale = 1.0 / np.sqrt(np.float32(D))
    wq_s = np.asarray(wq, np.float32) * scale
    bq_s = np.asarray(bq, np.float32) * scale
    wk = np.asarray(wkv[:E], np.float32)
    bk = np.asarray(bkv[:E], np.float32)
    wv = np.asarray(wkv[E:], np.float32)
    bv = np.asarray(bkv[E:], np.float32)
    wproj = np.asarray(wproj, np.float32)
    bproj_eff = (np.asarray(bproj, np.float32)
                 + wproj.astype(np.float64) @ bv.astype(np.float64)).astype(np.float32)

    shared = {
        "onesd": np.ones((8, H), ml_dtypes.bfloat16),
        "onr": np.ones((1, 32), np.float32),
        "wqt": np.ascontiguousarray(wq_s.T),
        "wkt": np.ascontiguousarray(wk.T),
        "wvt": np.ascontiguousarray(wv.T),
        "wpt": np.ascontiguousarray(wproj.T),
        "bqd": bq_s,
        "bkd": bk,
        "bpd": bproj_eff,
    }
    in_maps = [
        {"xin": np.ascontiguousarray(x[i].reshape(C, S)), **shared}
        for i in range(N_CORES)
    ]
    res = run_bass_kernel_spmd(nc, in_maps, core_ids=list(range(N_CORES)))
    out = np.stack([res.results[i]["out"].reshape(O, 32, 32) for i in range(N_CORES)])
    return out.astype(np.float32)



# revision 3
# speedup vs baseline: 1.1871x; 1.1871x over previous
"""Causal multi-head attention (8 heads, 1x1-conv projections) on 8 TRN2 cores.

Sharding: data-parallel over batch N=8 -> one batch element per NeuronCore.
Per-core kernel (S=1024 pixels, C=E=256 channels, H=8 heads, d=32), bf16
matmul inputs / fp32 accumulate:
  q = Wq^T x + bq (scaled by 1/sqrt(d) host-side), k = Wk^T x  (k bias is
  dropped: its score contribution is constant per query column, which
  softmax cancels)
  vT = x^T Wv  (s, e) layout with an appended ones column per head, so the
  softmax denominator falls out of the attn@v matmul (row 32 of psum)
  scores are computed TRANSPOSED, P^T[sk, sq], two heads per psum tile so
  one exp activation covers both heads of a chunk window (fewer ACT
  instructions -> less fixed overhead on the bottleneck engine)
  causal mask: gpsimd affine_select zeroes the upper triangle of each
  diagonal 128x128 block after exp (both heads in one op)
  normalization: DVE reciprocal of the denominator row -> gpsimd
  partition_broadcast across 32 partitions -> DVE multiply straight out of
  PSUM (no DMA bounce, no extra copy)
  out = Wproj^T att + bproj_eff (v bias folded through the projection)
Emission is software-pipelined across head pairs: scores/exp for pair t
interleave with attn@v + normalization of pair t-1, the first head-pair's
j0-half scores only need the first half of x (early exp start), and the
final head pair finishes in 256-column quarters so the last exp gates only
a short tail.
"""

import numpy as np

N_CORES = 8
C = 256      # input channels
E = 256      # embed channels (q/k)
O = 256      # v/out channels
S = 1024     # spatial positions (32*32)
H = 8        # heads
D = 32       # head dim
NCH = 2      # 256 = 2 * 128 partition chunks

_CACHE = {}


def _build_program():
    import concourse.bass as bass
    import concourse.mybir as mybir
    from concourse import bacc
    from concourse import library_config
    from concourse.tile import TileContext

    F32 = mybir.dt.float32
    BF16 = mybir.dt.bfloat16
    EXP = mybir.ActivationFunctionType.Exp

    nc = bacc.Bacc("TRN2", target_bir_lowering=False, debug=False)

    xin = nc.dram_tensor("xin", [C, S], BF16, kind="ExternalInput")
    wqk = nc.dram_tensor("wqk", [C, 2 * E], BF16, kind="ExternalInput")
    wvp = nc.dram_tensor("wvp", [C, 2 * O], BF16, kind="ExternalInput")
    onesd = nc.dram_tensor("onesd", [8, H], BF16, kind="ExternalInput")
    bqd = nc.dram_tensor("bqd", [E], F32, kind="ExternalInput")
    bpd = nc.dram_tensor("bpd", [O], F32, kind="ExternalInput")
    outd = nc.dram_tensor("out", [O, S], F32, kind="ExternalOutput")

    with TileContext(nc) as tc:
        with (
            tc.tile_pool(name="cst", bufs=1) as cst,
            tc.tile_pool(name="ptp", bufs=2) as ptp,
            tc.tile_pool(name="rbp", bufs=4) as rbp,
            tc.tile_pool(name="osb", bufs=2) as osb,
            tc.tile_pool(name="spp", bufs=2, space="PSUM") as spp,
            tc.tile_pool(name="pav", bufs=2, space="PSUM") as pav,
            tc.tile_pool(name="psc", bufs=2, space="PSUM") as psc,
        ):
            nc.gpsimd.load_library(library_config.attn)

            # hoist the exp table load off the critical path
            dmz = cst.tile([128, 8], F32, tag="dmz")
            nc.vector.memset(dmz, 0.0)
            dme = cst.tile([128, 1], F32, tag="dme")
            nc.scalar.activation(dme, dmz[:, 0:1], EXP)

            # --- input DMAs, ordered along the critical path ---
            wqk_sb = cst.tile([128, NCH, 2 * E], BF16, tag="wqk")
            nc.sync.dma_start(
                out=wqk_sb, in_=wqk.ap().rearrange("(c p) e -> p c e", p=128)
            )
            xr = cst.tile([128, NCH, S], BF16, tag="xr")
            xsrc = xin.ap().rearrange("(c p) s -> p c s", p=128)
            nc.sync.dma_start(out=xr[:, :, 0:512], in_=xsrc[:, :, 0:512])
            bq_t = cst.tile([128, NCH], F32, tag="bq")
            nc.sync.dma_start(out=bq_t, in_=bqd.ap().rearrange("(m p) -> p m", p=128))
            nc.sync.dma_start(out=xr[:, :, 512:1024], in_=xsrc[:, :, 512:1024])
            wvp_sb = cst.tile([128, NCH, 2 * O], BF16, tag="wvp")
            nc.sync.dma_start(
                out=wvp_sb, in_=wvp.ap().rearrange("(c p) e -> p c e", p=128)
            )
            bp_t = cst.tile([128, NCH], F32, tag="bp")
            nc.sync.dma_start(out=bp_t, in_=bpd.ap().rearrange("(m p) -> p m", p=128))
            # v^T augmented with a ones column per head: (sk_part, chunk, head, 33)
            vaug = cst.tile([128, 8, H, D + 1], BF16, tag="vaug")
            oap = onesd.ap()
            ones_bcast = bass.AP(
                tensor=oap.tensor, offset=oap.offset, ap=[[0, 128]] + list(oap.ap)
            )
            nc.sync.dma_start(out=vaug[:, :, :, D], in_=ones_bcast)

            q_sb = cst.tile([128, NCH, S], BF16, tag="q_sb")
            k_sb = cst.tile([128, NCH, S], BF16, tag="k_sb")
            att = cst.tile([128, NCH, S], BF16, tag="att")

            def qk_unit(which, m, j):
                pp = psc.tile([128, 512], F32, tag="sc")
                base = 0 if which == "q" else E
                for c in range(2):
                    nc.tensor.matmul(
                        pp,
                        wqk_sb[:, c, base + m * 128:base + (m + 1) * 128],
                        xr[:, c, j * 512:(j + 1) * 512],
                        start=(c == 0), stop=(c == 1),
                    )
                dst = (q_sb if which == "q" else k_sb)[:, m, j * 512:(j + 1) * 512]
                if which == "q":
                    nc.vector.tensor_scalar_add(dst, pp, bq_t[:, m:m + 1])
                else:
                    nc.vector.tensor_copy(dst, pp)

            def v_unit(i):
                pv = psc.tile([128, 512], F32, tag="sc")
                for c in range(2):
                    nc.tensor.matmul(
                        pv[:, 0:256],
                        xr[:, c, i * 128:(i + 1) * 128],
                        wvp_sb[:, c, 0:256],
                        start=(c == 0), stop=(c == 1),
                    )
                nc.vector.tensor_copy(
                    vaug[:, i, :, 0:D],
                    pv[:, 0:256].rearrange("p (h d) -> p h d", h=H),
                )

            pts_tiles = {}

            def sstep(t, i, j):
                """Scores + exp for head pair t, sk-chunk i, sq-window j."""
                m = t // 2
                ws, we = max(512 * j, 128 * i), 512 * (j + 1)
                spt = spp.tile([128, 2, 512], F32, tag="sp")
                for hh in range(2):
                    r = (2 * t + hh) % 4
                    nc.tensor.matmul(
                        spt[:, hh, ws - 512 * j:512],
                        k_sb[32 * r:32 * r + 32, m, 128 * i:128 * (i + 1)],
                        q_sb[32 * r:32 * r + 32, m, ws:we],
                        start=True, stop=True,
                        tile_position=(32 * r, 0),
                    )
                pts = pts_tiles[t]
                nc.scalar.activation(
                    pts[:, i, :, ws:we], spt[:, :, ws - 512 * j:512], EXP
                )
                if j == i // 4:
                    # zero the masked (sq_local < sk_local) part of the
                    # diagonal block, both heads at once
                    nc.gpsimd.affine_select(
                        out=pts[:, i, :, 128 * i:128 * (i + 1)],
                        in_=pts[:, i, :, 128 * i:128 * (i + 1)],
                        compare_op=mybir.AluOpType.is_ge,
                        fill=0.0,
                        base=0,
                        channel_multiplier=-1,
                        pattern=[[0, 2], [1, 128]],
                    )

            def attnv(h, c0, w):
                """attn@v + normalization for head h, sq columns [c0, c0+w)."""
                t, m, r = h // 2, h // 4, h % 4
                pts = pts_tiles[t]
                hh = h % 2
                pa = pav.tile([33, 512], F32, tag="pa")
                ii = [i for i in range(8) if 128 * i < c0 + w]
                for idx, i in enumerate(ii):
                    ws = max(c0, 128 * i)
                    nc.tensor.matmul(
                        pa[:, ws - c0:w],
                        vaug[:, i, h, :],
                        pts[:, i, hh, ws:c0 + w],
                        start=(idx == 0), stop=(idx == len(ii) - 1),
                    )
                rfd = rbp.tile([1, 512], F32, tag="rfd")
                nc.vector.reciprocal(rfd[:, 0:w], pa[32:33, 0:w])
                rb = rbp.tile([32, 512], F32, tag="rb")
                nc.gpsimd.partition_broadcast(rb[:, 0:w], rfd[:, 0:w], channels=32)
                nc.vector.tensor_mul(
                    att[32 * r:32 * r + 32, m, c0:c0 + w],
                    pa[0:32, 0:w], rb[:, 0:w],
                )

            def outproj(m, c0, w):
                po = psc.tile([128, 512], F32, tag="sc")
                for c in range(2):
                    nc.tensor.matmul(
                        po[:, 0:w],
                        wvp_sb[:, c, O + m * 128:O + (m + 1) * 128],
                        att[:, c, c0:c0 + w],
                        start=(c == 0), stop=(c == 1),
                    )
                ot = osb.tile([128, 512], F32, tag="ot")
                nc.vector.tensor_scalar_add(ot[:, 0:w], po[:, 0:w], bp_t[:, m:m + 1])
                nc.sync.dma_start(
                    out=outd.ap().rearrange("(m p) s -> p m s", p=128)[
                        :, m, c0:c0 + w
                    ],
                    in_=ot[:, 0:w],
                )

            # --- software-pipelined emission over head pairs ---
            qk_unit("q", 0, 0)
            qk_unit("k", 0, 0)

            # injections keyed by (pair, phase, step): lists of thunks
            inj = {
                (0, 0, 2): [lambda: qk_unit("q", 0, 1)],
                (0, 0, 3): [lambda: qk_unit("k", 0, 1)],
                (0, 1, 0): [lambda: v_unit(0), lambda: v_unit(1)],
                (0, 1, 1): [lambda: v_unit(2), lambda: v_unit(3),
                            lambda: attnv(0, 0, 512)],
                (0, 1, 2): [lambda: v_unit(4), lambda: v_unit(5)],
                (0, 1, 3): [lambda: v_unit(6), lambda: v_unit(7),
                            lambda: attnv(1, 0, 512)],
                (1, 0, 0): [lambda: attnv(0, 512, 512)],
                (1, 0, 1): [lambda: attnv(1, 512, 512)],
                (1, 0, 2): [lambda: qk_unit("q", 1, 0)],
                (1, 0, 3): [lambda: qk_unit("k", 1, 0)],
                (1, 1, 0): [lambda: qk_unit("q", 1, 1)],
                (1, 1, 1): [lambda: qk_unit("k", 1, 1),
                            lambda: attnv(2, 0, 512)],
                (1, 1, 3): [lambda: attnv(3, 0, 512)],
                (2, 0, 0): [lambda: attnv(2, 512, 512)],
                (2, 0, 1): [lambda: attnv(3, 512, 512)],
                (2, 1, 1): [lambda: attnv(4, 0, 512)],
                (2, 1, 3): [lambda: attnv(5, 0, 512)],
                (3, 0, 0): [lambda: attnv(4, 512, 512)],
                (3, 0, 1): [lambda: attnv(5, 512, 512)],
                (3, 1, 1): [lambda: attnv(6, 0, 512)],
                (3, 1, 2): [lambda: attnv(7, 0, 512)],
                (3, 1, 4): [lambda: outproj(0, 0, 512)],
                (3, 1, 5): [lambda: outproj(1, 0, 512),
                            lambda: attnv(6, 512, 256),
                            lambda: attnv(7, 512, 256)],
                (3, 1, 6): [lambda: outproj(0, 512, 256)],
                (3, 1, 7): [lambda: outproj(1, 512, 256)],
            }

            for t in range(4):
                pts = ptp.tile([128, 8, 2, S], BF16, tag="pts")
                pts_tiles[t] = pts
                for i in range(4):
                    sstep(t, i, 0)
                    for f in inj.get((t, 0, i), ()):
                        f()
                for i in range(8):
                    sstep(t, i, 1)
                    for f in inj.get((t, 1, i), ()):
                        f()
                if t >= 1:
                    pts_tiles.pop(t - 1)

            # tail: only the last 256 columns still depend on the final exps
            attnv(6, 768, 256)
            attnv(7, 768, 256)
            outproj(0, 768, 256)
            outproj(1, 768, 256)

    nc.compile()
    return nc


def get_program():
    if "nc" not in _CACHE:
        _CACHE["nc"] = _build_program()
    return _CACHE["nc"]


def kernel(x, wq, bq, wkv, bkv, wproj, bproj):
    import ml_dtypes
    from concourse.bass_utils import run_bass_kernel_spmd

    nc = get_program()

    x = np.asarray(x, dtype=np.float32)
    n = x.shape[0]
    assert n == N_CORES and x.shape[1:] == (C, 32, 32)

    scale = 1.0 / np.sqrt(np.float32(D))
    wq_s = np.asarray(wq, np.float32) * scale
    bq_s = np.asarray(bq, np.float32) * scale
    wk = np.asarray(wkv[:E], np.float32)
    wv = np.asarray(wkv[E:], np.float32)
    bv = np.asarray(bkv[E:], np.float32)
    wproj = np.asarray(wproj, np.float32)
    bproj_eff = (np.asarray(bproj, np.float32)
                 + wproj.astype(np.float64) @ bv.astype(np.float64)).astype(np.float32)

    bf = ml_dtypes.bfloat16
    shared = {
        "onesd": np.ones((8, H), bf),
        "wqk": np.ascontiguousarray(
            np.concatenate([wq_s.T, wk.T], axis=1)).astype(bf),
        "wvp": np.ascontiguousarray(
            np.concatenate([wv.T, wproj.T], axis=1)).astype(bf),
        "bqd": bq_s,
        "bpd": bproj_eff,
    }
    in_maps = [
        {"xin": np.ascontiguousarray(x[i].reshape(C, S)).astype(bf), **shared}
        for i in range(N_CORES)
    ]
    res = run_bass_kernel_spmd(nc, in_maps, core_ids=list(range(N_CORES)))
    out = np.stack([res.results[i]["out"].reshape(O, 32, 32) for i in range(N_CORES)])
    return out.astype(np.float32)
